# revision 18
# baseline (speedup 1.0000x reference)
"""ChebyKAN layer on 8 Trainium2 NeuronCores.

y = einsum('dbi,dio->bo', cheby_basis(tanh(x)), cheby_coeffs)

Strategy (per core, data-parallel over batch):
  - each core takes 1024 rows of x (8192/8) and the full coeffs
  - x arrives pre-transposed ([i, b] layout) from the host; tanh on the
    scalar engine
  - Chebyshev basis built on-the-fly in fp32 on the vector engine,
    rounded to fp32r on the scalar engine
  - contraction as fp32r matmuls (full-rate on TRN2, ~1e-4 rel err):
    stationary = W[d, i-tile, o-tile], moving = T_d[i-tile, b-half],
    psum holds y.T chunks [o-tile 128, b-half 512] x 8 o-tiles = 8 banks
  - two b-halves of 512; W streamed from HBM once per half, cast to
    fp32r on the vector engine
  - output is y.T per core; host transposes and concatenates
"""

import numpy as np

import concourse.bass as bass
import concourse.tile as tile
from concourse import bacc, mybir
from concourse import bass_utils
from concourse.alu_op_type import AluOpType

N_CORES = 8
B = 8192
IC = 1024
OC = 1024
DEG = 8  # polynomial degree; DEG+1 = 9 basis terms
BC = B // N_CORES  # 1024 batch rows per core
P = 128
NI = IC // P  # 8 i-tiles
NO = OC // P  # 8 o-tiles
BH = BC // 2  # 512, b-half
F32 = mybir.dt.float32
F32R = mybir.dt.float32r

# W slab granularity (in i-tiles) per degree: the first degree of a half
# uses small slabs so the first matmul's W-DMA + cast chain is short.
_D0_SLABS = [1, 1, 2, 2, 2]
_D_SLABS = [2, 2, 2, 2]


def _build(tanh_scale: float, tanh_bias: float):
    nc = bacc.Bacc("TRN2", target_bir_lowering=False, debug=False, num_devices=N_CORES)

    xT_d = nc.dram_tensor("xT", [IC, BC], F32, kind="ExternalInput").ap()
    w_d = nc.dram_tensor("w", [DEG + 1, IC, OC], F32, kind="ExternalInput").ap()
    yt_d = nc.dram_tensor("yt", [OC, BC], F32, kind="ExternalOutput").ap()

    with tile.TileContext(nc) as tc:
        with (
            tc.tile_pool(name="const", bufs=1) as constp,
            tc.tile_pool(name="xin", bufs=3) as xinp,
            tc.tile_pool(name="xt", bufs=2) as xtp,
            tc.tile_pool(name="state", bufs=3) as statep,
            tc.tile_pool(name="prod", bufs=2) as prodp,
            tc.tile_pool(name="tr", bufs=2) as trp,
            tc.tile_pool(name="wstage", bufs=3) as wstagep,
            tc.tile_pool(name="wr", bufs=3) as wrp,
            tc.tile_pool(name="evac", bufs=2) as evacp,
            tc.tile_pool(name="ps", bufs=8, space=bass.MemorySpace.PSUM) as psp,
        ):
            ones_f = constp.tile([P, BH], F32)
            nc.vector.memset(ones_f[:], 1.0)
            ones_r = constp.tile([P, BH], F32R)
            nc.vector.tensor_copy(ones_r[:], ones_f[:])

            def emit_w_slabs(h, d, slab_sizes):
                """DMA W[d] i-tile slabs and cast to fp32r; returns
                [(first_it, ntiles, wr_tile), ...]."""
                out = []
                it0 = 0
                for ws, nt in enumerate(slab_sizes):
                    wst = wstagep.tile(
                        [P, nt * OC], F32, tag="wstage", name=f"wst_{h}_{d}_{ws}"
                    )
                    nc.sync.dma_start(
                        wst[:].rearrange("p (il o) -> p il o", il=nt),
                        w_d[d, it0 * P : (it0 + nt) * P, :].rearrange(
                            "(il p) o -> p il o", p=P
                        ),
                    )
                    wr = wrp.tile([P, nt * OC], F32R, tag="wr", name=f"wr_{h}_{d}_{ws}")
                    nc.vector.tensor_copy(wr[:], wst[:])
                    out.append((it0, nt, wr))
                    it0 += nt
                return out

            def emit_matmuls(accs, wr_slabs, d, tr_d):
                for si, (it0, nt, wr) in enumerate(wr_slabs):
                    last_slab = d == DEG and si == len(wr_slabs) - 1
                    if last_slab:
                        # ot-major: each acc's accumulation closes early so
                        # psum banks free progressively for the next half
                        for ot in range(NO):
                            for il in range(nt):
                                it = it0 + il
                                rhs = tr_d[:, it * BH : (it + 1) * BH]
                                nc.tensor.matmul(
                                    accs[ot][:],
                                    wr[:, il * OC + ot * P : il * OC + (ot + 1) * P],
                                    rhs,
                                    start=False,
                                    stop=(it == NI - 1),
                                )
                        continue
                    for il in range(nt):
                        it = it0 + il
                        if d == 0:
                            rhs = ones_r[:]
                        else:
                            rhs = tr_d[:, it * BH : (it + 1) * BH]
                        for ot in range(NO):
                            nc.tensor.matmul(
                                accs[ot][:],
                                wr[:, il * OC + ot * P : il * OC + (ot + 1) * P],
                                rhs,
                                start=(d == 0 and it == 0),
                                stop=(d == DEG and it == NI - 1),
                            )

            # W for the very first degree goes ahead of everything so the
            # PE starts as early as possible.
            d0_slabs_h0 = emit_w_slabs(0, 0, _D0_SLABS)

            # ---- x.T load + tanh -> fp32 xt, emitted per half ----
            # xt free layout: i_tile-major, 512 b-local each
            def emit_xt(h):
                xt = xtp.tile([P, NI * BH], F32, tag="xt", name=f"xt_{h}")
                for it0 in range(0, NI, 2):
                    xst = xinp.tile([P, 2 * BH], F32, tag="xin", name=f"xs_{h}_{it0}")
                    nc.scalar.dma_start(
                        xst[:].rearrange("p (il b) -> p il b", il=2),
                        xT_d[it0 * P : (it0 + 2) * P, h * BH : (h + 1) * BH].rearrange(
                            "(il p) b -> p il b", p=P
                        ),
                    )
                    nc.scalar.activation(
                        xt[:, it0 * BH : (it0 + 2) * BH],
                        xst[:],
                        mybir.ActivationFunctionType.Tanh,
                        bias=tanh_bias,
                        scale=tanh_scale,
                    )
                return xt

            xts = [emit_xt(0), None]

            for h in range(2):
                xt = xts[h]
                # ---- accumulation psum tiles: y.T chunk per o-tile ----
                accs = [
                    psp.tile([P, BH], F32, tag="ps", name=f"acc_h{h}_o{ot}")
                    for ot in range(NO)
                ]

                # ---- degree loop ----
                t_m1 = xt  # T_{d-1} (fp32 slab)
                t_m2 = None  # T_{d-2}
                for d in range(DEG + 1):
                    # fp32r moving operand for this degree
                    if d == 0:
                        tr_d = None  # use ones_r
                    elif d == 1:
                        tr_d = trp.tile([P, NI * BH], F32R, tag="tr", name=f"tr_{h}_1")
                        QS = NI * BH // 4
                        for q in range(4):
                            sl = slice(q * QS, (q + 1) * QS)
                            nc.scalar.activation(
                                tr_d[:, sl], xt[:, sl], mybir.ActivationFunctionType.Copy
                            )
                    else:
                        t_new = statep.tile(
                            [P, NI * BH], F32, tag="state", name=f"st_{h}_{d}"
                        )
                        tr_d = trp.tile([P, NI * BH], F32R, tag="tr", name=f"tr_{h}_{d}")
                        QS = NI * BH // 4
                        for q in range(4):
                            sl = slice(q * QS, (q + 1) * QS)
                            prod = prodp.tile(
                                [P, QS], F32, tag="prod", name=f"prod_{h}_{d}_{q}"
                            )
                            nc.vector.scalar_tensor_tensor(
                                prod[:],
                                t_m1[:, sl],
                                2.0,
                                xt[:, sl],
                                AluOpType.mult,
                                AluOpType.mult,
                            )
                            if d == 2:
                                # T2 = 2*xt^2 - 1
                                nc.vector.tensor_scalar_sub(t_new[:, sl], prod[:], 1.0)
                            else:
                                nc.vector.tensor_sub(t_new[:, sl], prod[:], t_m2[:, sl])
                            nc.scalar.activation(
                                tr_d[:, sl],
                                t_new[:, sl],
                                mybir.ActivationFunctionType.Copy,
                            )
                        t_m2, t_m1 = t_m1, t_new
                    if d == 1:
                        t_m2, t_m1 = xt, xt  # T1 = xt; T0 handled via scalar sub at d=2

                    # ---- W stream + matmuls for this degree ----
                    if h == 0 and d == 0:
                        wr_slabs = d0_slabs_h0
                    else:
                        wr_slabs = emit_w_slabs(h, d, _D0_SLABS if d == 0 else _D_SLABS)
                    emit_matmuls(accs, wr_slabs, d, tr_d)
                    if h == 0 and d == 2:
                        xts[1] = emit_xt(1)

                # ---- evacuate psum -> SBUF -> y.T ----
                for ot in range(NO):
                    ev = evacp.tile([P, BH], F32, tag="evac", name=f"ev_h{h}_o{ot}")
                    if ot % 2 == 0:
                        nc.vector.tensor_copy(ev[:], accs[ot][:])
                    else:
                        nc.scalar.activation(
                            ev[:], accs[ot][:], mybir.ActivationFunctionType.Copy
                        )
                    nc.scalar.dma_start(
                        yt_d[ot * P : (ot + 1) * P, h * BH : (h + 1) * BH],
                        ev[:],
                    )

    nc.compile()
    return nc


_CACHE: dict = {}


def make_in_maps(x, w):
    return [
        {"xT": np.ascontiguousarray(x[c * BC : (c + 1) * BC].T), "w": w}
        for c in range(N_CORES)
    ]


def kernel(x, cheby_coeffs, tanh_scale, tanh_bias):
    x = np.ascontiguousarray(np.asarray(x, dtype=np.float32))
    w = np.ascontiguousarray(np.asarray(cheby_coeffs, dtype=np.float32))
    ts = float(np.asarray(tanh_scale))
    tb = float(np.asarray(tanh_bias))

    key = (ts, tb)
    if key not in _CACHE:
        _CACHE[key] = _build(ts, tb)
    nc = _CACHE[key]

    in_maps = make_in_maps(x, w)
    res = bass_utils.run_bass_kernel_spmd(
        nc, in_maps, core_ids=list(range(N_CORES)), trace=False
    )

    y = np.empty((B, OC), dtype=np.float32)
    for c in range(N_CORES):
        y[c * BC : (c + 1) * BC, :] = res.results[c]["yt"].T
    return y
